# revision 38
# baseline (speedup 1.0000x reference)
"""Causal single-head attention (batch=8, ctx=2048, d=1024) on 8 trn2 cores.

Strategy: pure data-parallel over batch — core b computes attention for
batch element b with no cross-core communication.

Per-core pipeline:
  phase 1: Q^T, K^T (o-major) and V (s-major) projections accumulated in
           PSUM (fp32), consumed per 512-column s-group of x^T.
  phase 2: flash-style causal attention per 128-row q-block:
           S = Q^T.T @ K^T, additive causal mask on the diagonal
           128x128 sub-tile, one-pass softmax (per-tile exp(s - m_tile)
           on ACT with row-sum accumulators, exp(m_tile - m_all)
           correction folded into P), P transposed per tile on the PE,
           O = P @ V accumulated in PSUM, deferred normalization by the
           reciprocal row sum, DMA out (fp32); two-block software
           pipeline so PV of one block hides the next one's softmax.

MODE selects the matmul input dtype:
  "fp32": all matmul inputs fp32 (4 cyc/row); x^T built on-device via PE
          transposes; Q^T/K^T/V staged through DRAM scratch (SBUF can't
          hold x^T + all three in fp32).  ~1.25ms, rel err ~6e-6.
  "fp16": matmul inputs fp16 (1 cyc/row), fp32 PSUM accumulation and
          softmax; x^T and the weights are pre-cast/pre-transposed on the
          host (pure layout prep, bit-identical to a DVE cast) and
          everything stays resident in SBUF.  ~332us, rel err ~5e-4.
"""

import sys

sys.path.insert(0, "/opt/trn_rl_repo")

import numpy as np

import concourse.bass as bass
import concourse.mybir as mybir
import concourse.tile as tile
from concourse.masks import make_identity
from concourse.vector_clock import ScopedClock

MODE = "fp16"

BATCH = 8
CTX = 2048
D_IN = 1024
D_OUT = 1024
N_CORES = 8
P = 128
F32 = mybir.dt.float32
F16 = mybir.dt.float16
NEG_BIG = -1.0e30

# ---------------------------------------------------------------------------
# Workarounds for the walrus build on this stack: it rejects any instruction
# carrying more than ONE sync wait.  (1) Patch the TileContext tail drain to
# spread its waits over preceding sync-engine nops; (2) post-pass that hoists
# extra waits from any instruction onto same-engine nops inserted right
# before it (sequencers execute per-engine streams in order, so this is
# semantics-preserving).
# ---------------------------------------------------------------------------


def _patched_drain_and_barrier(self, tick_clock, wait_clock):
    nc = self.nc
    nops = [nc.sync.nop(nofuse=True) for _ in range(27)]
    drain_inst = nc.sync.drain()
    wait_clock.add_sem_waits(
        drain_inst.ins, ScopedClock({None: tick_clock.global_clock})
    )
    si = drain_inst.ins.sync_info
    if si is not None and si.on_wait is not None and len(si.on_wait) > 1:
        waits = list(si.on_wait)
        si.on_wait = waits[:1]
        rest = waits[1:]
        for i, nop in enumerate(nops):
            chunk = rest[i : i + 1]
            if not chunk:
                break
            nsi = nop.ins.sync_info
            if nsi is None:
                nop.ins.sync_info = mybir.SyncInfo(on_wait=chunk, on_update=[])
            else:
                nsi.on_wait = chunk

    nc.all_engine_barrier()
    assert self.sems is not None
    popped = nc._tile_sem_poison_stack.pop()
    assert popped is self._sem_poison
    nc.clear_and_free_semaphores(list(self.sems.allocated().values()))
    nc.all_engine_barrier()


tile.TileContext._drain_and_barrier = _patched_drain_and_barrier


def _split_multi_waits(nc):
    n_split = 0
    for f in nc.m.functions:
        for bb in f.blocks:
            il = bb.instructions
            if not any(
                inst.sync_info is not None
                and inst.sync_info.on_wait
                and len(inst.sync_info.on_wait) > 1
                for inst in il
            ):
                continue
            new = []
            for inst in il:
                si = inst.sync_info
                if si is not None and si.on_wait and len(si.on_wait) > 1:
                    waits = list(si.on_wait)
                    for w in waits[:-1]:
                        nop = mybir.InstNoOp(
                            name=nc.get_next_instruction_name(), ins=[], outs=[]
                        )
                        nop.engine = inst.engine
                        nop.sync_info = mybir.SyncInfo(on_wait=[w], on_update=[])
                        new.append(nop)
                        n_split += 1
                    si.on_wait = [waits[-1]]
                new.append(inst)
            il[:] = new
    return n_split


# ---------------------------------------------------------------------------
# Program builders
# ---------------------------------------------------------------------------

IC = D_IN // P  # 8 input-dim chunks
OC = D_OUT // P  # 8 output-dim chunks
ST = CTX // P  # 16 seq chunks
QB = CTX // P  # 16 q blocks
MMW = 512  # moving width (psum bank = 512 fp32)


def _declare_io(nc):
    x_d = nc.declare_dram_parameter("x", [CTX, D_IN], F32, isOutput=False)
    wq_d = nc.declare_dram_parameter("Wq", [D_IN, D_OUT], F32, isOutput=False)
    wk_d = nc.declare_dram_parameter("Wk", [D_IN, D_OUT], F32, isOutput=False)
    wv_d = nc.declare_dram_parameter("Wv", [D_IN, D_OUT], F32, isOutput=False)
    negmask_d = nc.declare_dram_parameter("negmask", [P, P], F32, isOutput=False)
    out_d = nc.declare_dram_parameter("out", [CTX, D_OUT], F32, isOutput=True)
    return x_d, wq_d, wk_d, wv_d, negmask_d, out_d


def _attention_phase(nc, tc, consts_ident, negmask, qt_src, kt_sb, v_sb, out_d, dt):
    """qt_src(qb) -> [P, OC, P] tile of Q^T for that block.

    One-pass softmax: each score tile gets exp(s - m_tile) immediately
    (ACT, off the PE critical path); after the block's tiles are done the
    per-tile correction exp(m_tile - m_all) is folded into the 16-bit P
    tiles and the row-sum."""
    with (
        tc.tile_pool(name="pexp", bufs=3) as p_pool,
        tc.tile_pool(name="pexp32", bufs=3) as p32_pool,
        tc.tile_pool(name="ptr", bufs=2) as pt_pool,
        tc.tile_pool(name="red", bufs=3) as red_pool,
        tc.tile_pool(name="ob", bufs=3) as o_pool,
        tc.tile_pool(name="ps_s", bufs=4, space="PSUM") as ps_s,
        tc.tile_pool(name="ps_o", bufs=2, space="PSUM") as ps_o,
        tc.tile_pool(name="ps_pt", bufs=2, space="PSUM") as ps_pt,
    ):

        def emit_scores(qb):
            L = (qb + 1) * P
            ktiles = [(k0, min(MMW, L - k0)) for k0 in range(0, L, MMW)]
            nkt = len(ktiles)

            qt_b = qt_src(qb)

            red = red_pool.tile([P, 4 * nkt + 3], F32, tag="red")
            mx = red[:, 0:nkt]
            negm = red[:, nkt : 2 * nkt]
            sm = red[:, 2 * nkt : 3 * nkt]
            scl = red[:, 3 * nkt : 4 * nkt]
            negm_all = red[:, 4 * nkt : 4 * nkt + 1]
            rsum = red[:, 4 * nkt + 1 : 4 * nkt + 2]
            rinv = red[:, 4 * nkt + 2 : 4 * nkt + 3]

            p_sb = p_pool.tile([P, CTX], dt, tag="p")
            p32_sb = p32_pool.tile([P, CTX], F32, tag="p32")
            for idx, (k0, w) in enumerate(ktiles):
                ps = ps_s.tile([P, MMW], F32, tag="ps_s")
                for oc in range(OC):
                    nc.tensor.matmul(
                        ps[:, :w],
                        qt_b[:, oc, :],
                        kt_sb[:, oc, k0 : k0 + w],
                        start=(oc == 0),
                        stop=(oc == OC - 1),
                    )
                if idx == nkt - 1:
                    nc.vector.tensor_add(
                        ps[:, w - P : w], ps[:, w - P : w], negmask[:]
                    )
                nc.vector.reduce_max(
                    mx[:, idx : idx + 1], ps[:, :w], axis=mybir.AxisListType.X
                )
                nc.scalar.mul(
                    negm[:, idx : idx + 1], mx[:, idx : idx + 1], -0.03125
                )
                # exp(s - m_tile) immediately; row-sums into sm[idx]
                nc.scalar.activation(
                    p32_sb[:, k0 : k0 + w],
                    ps[:, :w],
                    mybir.ActivationFunctionType.Exp,
                    bias=negm[:, idx : idx + 1],
                    scale=0.03125,
                    accum_out=sm[:, idx : idx + 1],
                )
            # combine: negm_all = min_idx(-m_idx/32) = -m_all/32;
            # scl_idx = exp((m_idx - m_all)/32)
            nc.vector.tensor_reduce(
                negm_all[:], negm[:], axis=mybir.AxisListType.X,
                op=mybir.AluOpType.min,
            )
            nc.scalar.activation(
                scl[:],
                mx[:],
                mybir.ActivationFunctionType.Exp,
                bias=negm_all[:, 0:1],
                scale=0.03125,
            )
            nc.vector.tensor_mul(sm[:], sm[:], scl[:])
            nc.vector.reduce_sum(rsum[:], sm[:], axis=mybir.AxisListType.X)
            nc.vector.reciprocal(rinv[:], rsum[:])
            for idx, (k0, w) in enumerate(ktiles):
                nc.vector.tensor_scalar_mul(
                    p_sb[:, k0 : k0 + w],
                    p32_sb[:, k0 : k0 + w],
                    scl[:, idx : idx + 1],
                )
            return {"qb": qb, "p_sb": p_sb, "rinv": rinv}

        def emit_pv(stt):
            qb = stt["qb"]
            p_sb = stt["p_sb"]
            rinv = stt["rinv"]
            L = (qb + 1) * P
            pt_sb = pt_pool.tile([P, L], dt, tag="pt")
            for c0 in range(0, qb + 1, 4):
                cn = min(4, qb + 1 - c0)
                ptp = ps_pt.tile([P, 4 * P], dt, tag="ptp")
                for j in range(cn):
                    kc = c0 + j
                    nc.tensor.transpose(
                        ptp[:, j * P : (j + 1) * P],
                        p_sb[:, kc * P : (kc + 1) * P],
                        consts_ident[:],
                    )
                nc.vector.tensor_copy(
                    pt_sb[:, c0 * P : c0 * P + cn * P], ptp[:, : cn * P]
                )

            o_sb = o_pool.tile([P, D_OUT], F32, tag="o")
            for o0 in range(0, D_OUT, MMW):
                pso = ps_o.tile([P, MMW], F32, tag="ps_o")
                for kc in range(qb + 1):
                    nc.tensor.matmul(
                        pso[:],
                        pt_sb[:, kc * P : (kc + 1) * P],
                        v_sb[:, kc, o0 : o0 + MMW],
                        start=(kc == 0),
                        stop=(kc == qb),
                    )
                nc.vector.tensor_scalar_mul(
                    o_sb[:, o0 : o0 + MMW], pso[:], rinv[:, 0:1]
                )
            nc.sync.dma_start(out_d[qb * P : (qb + 1) * P, :], o_sb[:])

        # two-block software pipeline: PV of the previous block hides the
        # softmax latency of the current one.  The four smallest blocks run
        # first (their PV is too short to hide a softmax), then largest-
        # first, so the exposed tail block still has a few-us PV.
        order = [3, 2, 1, 0] + list(range(QB - 1, 3, -1))
        prev = None
        for qb in order:
            stt = emit_scores(qb)
            if prev is not None:
                emit_pv(prev)
            prev = stt
        emit_pv(prev)


def build_program_fp32():
    nc = bass.Bass()
    x_d, wq_d, wk_d, wv_d, negmask_d, out_d = _declare_io(nc)
    qt_d = nc.dram_tensor("qt_scratch", [D_OUT, CTX], F32)
    kt_d = nc.dram_tensor("kt_scratch", [D_OUT, CTX], F32)
    v_d = nc.dram_tensor("v_scratch", [CTX, D_OUT], F32)

    with tile.TileContext(nc) as tc:
        with tc.tile_pool(name="consts", bufs=1) as consts:
            ident = consts.tile([P, P], F32)
            make_identity(nc, ident[:])
            negmask = consts.tile([P, P], F32)
            nc.sync.dma_start(negmask[:], negmask_d[:])

            with (
                tc.tile_pool(name="xt", bufs=1) as xt_pool,
                tc.tile_pool(name="w", bufs=2) as w_pool,
                tc.tile_pool(name="xs", bufs=3) as xs_pool,
                tc.tile_pool(name="stage", bufs=4) as stage_pool,
                tc.tile_pool(name="ps_proj", bufs=4, space="PSUM") as ps_proj,
                tc.tile_pool(name="ps_tr", bufs=4, space="PSUM") as ps_tr,
            ):
                xt = xt_pool.tile([P, IC, CTX], F32)
                for st in range(ST):
                    xs = xs_pool.tile([P, D_IN], F32, tag="xs")
                    nc.sync.dma_start(xs[:], x_d[st * P : (st + 1) * P, :])
                    for ic in range(IC):
                        pt = ps_tr.tile([P, P], F32, tag="pt")
                        nc.tensor.transpose(
                            pt[:], xs[:, ic * P : (ic + 1) * P], ident[:]
                        )
                        nc.vector.tensor_copy(xt[:, ic, st * P : (st + 1) * P], pt[:])

                for w_d, dst in ((wq_d, qt_d), (wk_d, kt_d)):
                    w_sb = w_pool.tile([P, IC, D_OUT], F32, tag="w")
                    nc.sync.dma_start(
                        w_sb[:], w_d[:].rearrange("(c p) o -> p c o", p=P)
                    )
                    for s0 in range(0, CTX, MMW):
                        for oc in range(OC):
                            ps = ps_proj.tile([P, MMW], F32, tag="ps")
                            for ic in range(IC):
                                nc.tensor.matmul(
                                    ps[:],
                                    w_sb[:, ic, oc * P : (oc + 1) * P],
                                    xt[:, ic, s0 : s0 + MMW],
                                    start=(ic == 0),
                                    stop=(ic == IC - 1),
                                )
                            sg = stage_pool.tile([P, MMW], F32, tag="sg")
                            nc.vector.tensor_copy(sg[:], ps[:])
                            nc.sync.dma_start(
                                dst[oc * P : (oc + 1) * P, s0 : s0 + MMW], sg[:]
                            )

                wv_sb = w_pool.tile([P, IC, D_OUT], F32, tag="w")
                nc.sync.dma_start(
                    wv_sb[:], wv_d[:].rearrange("(c p) o -> p c o", p=P)
                )
                for st in range(ST):
                    for o0 in range(0, D_OUT, MMW):
                        ps = ps_proj.tile([P, MMW], F32, tag="ps")
                        for ic in range(IC):
                            nc.tensor.matmul(
                                ps[:],
                                xt[:, ic, st * P : (st + 1) * P],
                                wv_sb[:, ic, o0 : o0 + MMW],
                                start=(ic == 0),
                                stop=(ic == IC - 1),
                            )
                        sg = stage_pool.tile([P, MMW], F32, tag="sg")
                        nc.vector.tensor_copy(sg[:], ps[:])
                        nc.sync.dma_start(
                            v_d[st * P : (st + 1) * P, o0 : o0 + MMW], sg[:]
                        )

            with (
                tc.tile_pool(name="kt", bufs=1) as kt_pool,
                tc.tile_pool(name="v", bufs=1) as v_pool,
                tc.tile_pool(name="qtb", bufs=2) as qtb_pool,
            ):
                kt_sb = kt_pool.tile([P, OC, CTX], F32)
                for c in range(OC):
                    nc.sync.dma_start(kt_sb[:, c, :], kt_d[c * P : (c + 1) * P, :])
                v_sb = v_pool.tile([P, ST, D_OUT], F32)
                for c in range(ST):
                    nc.sync.dma_start(v_sb[:, c, :], v_d[c * P : (c + 1) * P, :])

                def qt_src(qb):
                    qt_b = qtb_pool.tile([P, OC, P], F32, tag="qtb")
                    for oc in range(OC):
                        nc.sync.dma_start(
                            qt_b[:, oc, :],
                            qt_d[oc * P : (oc + 1) * P, qb * P : (qb + 1) * P],
                        )
                    return qt_b

                _attention_phase(
                    nc, tc, ident, negmask, qt_src, kt_sb, v_sb, out_d, F32
                )

    _split_multi_waits(nc)
    return nc


def build_program_fp16():
    """fp16 build: x^T and the weights are pre-cast/pre-transposed to fp16 on
    the HOST (pure layout prep; identical round-to-nearest as a DVE cast), so
    the device only does matmuls, softmax and the P transposes."""
    nc = bass.Bass()
    xt_d = nc.declare_dram_parameter("xT16", [D_IN, CTX], F16, isOutput=False)
    wq_d = nc.declare_dram_parameter("Wq16", [D_IN, D_OUT], F16, isOutput=False)
    wk_d = nc.declare_dram_parameter("Wk16", [D_IN, D_OUT], F16, isOutput=False)
    wv_d = nc.declare_dram_parameter("Wv16", [D_IN, D_OUT], F16, isOutput=False)
    negmask_d = nc.declare_dram_parameter("negmask", [P, P], F32, isOutput=False)
    out_d = nc.declare_dram_parameter("out", [CTX, D_OUT], F32, isOutput=True)

    with tile.TileContext(nc) as tc:
        with tc.tile_pool(name="consts", bufs=1) as consts:
            ident16 = consts.tile([P, P], F16)
            make_identity(nc, ident16[:])
            negmask = consts.tile([P, P], F32)
            nc.sync.dma_start(negmask[:], negmask_d[:])

            with (
                tc.tile_pool(name="qt", bufs=1) as qt_pool,
                tc.tile_pool(name="kt", bufs=1) as kt_pool,
                tc.tile_pool(name="v", bufs=1) as v_pool,
            ):
                qt_sb = qt_pool.tile([P, OC, CTX], F16)
                kt_sb = kt_pool.tile([P, OC, CTX], F16)
                v_sb = v_pool.tile([P, ST, D_OUT], F16)

                with (
                    tc.tile_pool(name="xt", bufs=1) as xt_pool,
                    tc.tile_pool(name="w", bufs=1) as w_pool,
                    tc.tile_pool(name="ps_proj", bufs=8, space="PSUM") as ps_proj,
                ):
                    # x^T arrives per (i-chunk, 512-col s-group); group-0
                    # chunks are queued BEFORE the weight loads so the first
                    # projection group only waits for ~1MB of x^T + 2MB of Wq.
                    xt = xt_pool.tile([P, IC, CTX], F16)
                    SG = MMW // P  # stripes per s-group

                    def load_xt_group(g):
                        s0 = g * MMW
                        for ic in range(IC):
                            nc.sync.dma_start(
                                xt[:, ic, s0 : s0 + MMW],
                                xt_d[ic * P : (ic + 1) * P, s0 : s0 + MMW],
                            )

                    wq_sb = w_pool.tile([P, IC, D_OUT], F16, tag="wq")
                    nc.sync.dma_start(wq_sb[:, 0, :], wq_d[0:P, :])
                    load_xt_group(0)
                    wk_sb = w_pool.tile([P, IC, D_OUT], F16, tag="wk")
                    wv_sb = w_pool.tile([P, IC, D_OUT], F16, tag="wv")
                    for ic in range(1, IC):
                        nc.sync.dma_start(
                            wq_sb[:, ic, :], wq_d[ic * P : (ic + 1) * P, :]
                        )
                        nc.sync.dma_start(
                            wk_sb[:, ic - 1, :], wk_d[(ic - 1) * P : ic * P, :]
                        )
                    nc.sync.dma_start(wk_sb[:, IC - 1, :], wk_d[(IC - 1) * P :, :])
                    for ic in range(IC):
                        nc.sync.dma_start(
                            wv_sb[:, ic, :], wv_d[ic * P : (ic + 1) * P, :]
                        )

                    for g in range(ST // SG):
                        s0 = g * MMW
                        if g > 0:
                            load_xt_group(g)
                        for dst, w_sb in ((qt_sb, wq_sb), (kt_sb, wk_sb)):
                            for oc in range(OC):
                                ps = ps_proj.tile([P, MMW], F32, tag="ps")
                                for ic in range(IC):
                                    nc.tensor.matmul(
                                        ps[:],
                                        w_sb[:, ic, oc * P : (oc + 1) * P],
                                        xt[:, ic, s0 : s0 + MMW],
                                        start=(ic == 0),
                                        stop=(ic == IC - 1),
                                    )
                                nc.vector.tensor_copy(
                                    dst[:, oc, s0 : s0 + MMW], ps[:]
                                )
                        for st in range(g * SG, (g + 1) * SG):
                            for o0 in range(0, D_OUT, MMW):
                                ps = ps_proj.tile([P, MMW], F32, tag="ps")
                                for ic in range(IC):
                                    nc.tensor.matmul(
                                        ps[:],
                                        xt[:, ic, st * P : (st + 1) * P],
                                        wv_sb[:, ic, o0 : o0 + MMW],
                                        start=(ic == 0),
                                        stop=(ic == IC - 1),
                                    )
                                nc.vector.tensor_copy(
                                    v_sb[:, st, o0 : o0 + MMW], ps[:]
                                )

                def qt_src(qb):
                    return qt_sb[:, :, qb * P : (qb + 1) * P]

                _attention_phase(
                    nc, tc, ident16, negmask, qt_src, kt_sb, v_sb, out_d, F16
                )

    _split_multi_waits(nc)
    return nc


_program_cache = {}


def build_program(mode=None):
    mode = mode or MODE
    if mode == "fp32":
        return build_program_fp32()
    elif mode == "fp16":
        return build_program_fp16()
    raise ValueError(mode)


def make_in_maps(x, Wq, Wk, Wv):
    x = np.ascontiguousarray(np.asarray(x), dtype=np.float32)
    Wq = np.ascontiguousarray(np.asarray(Wq), dtype=np.float32)
    Wk = np.ascontiguousarray(np.asarray(Wk), dtype=np.float32)
    Wv = np.ascontiguousarray(np.asarray(Wv), dtype=np.float32)

    iu = np.triu(np.ones((P, P), dtype=np.float32), k=1)
    negmask = (iu * NEG_BIG).astype(np.float32)

    if MODE == "fp16":
        # host-side layout prep: fp16 round-to-nearest (same as a DVE cast)
        # and the x transpose the device would otherwise do on the PE
        xT16 = np.ascontiguousarray(
            np.transpose(x.astype(np.float16), (0, 2, 1))
        )
        wq16 = np.ascontiguousarray(Wq.astype(np.float16))
        wk16 = np.ascontiguousarray(Wk.astype(np.float16))
        wv16 = np.ascontiguousarray(Wv.astype(np.float16))
        in_maps = [
            {
                "xT16": xT16[b],
                "Wq16": wq16,
                "Wk16": wk16,
                "Wv16": wv16,
                "negmask": negmask,
            }
            for b in range(BATCH)
        ]
    else:
        in_maps = [
            {"x": x[b], "Wq": Wq, "Wk": Wk, "Wv": Wv, "negmask": negmask}
            for b in range(BATCH)
        ]
    return in_maps


def kernel(x, Wq, Wk, Wv):
    from concourse.bass_utils import run_bass_kernel_spmd

    if MODE not in _program_cache:
        _program_cache[MODE] = build_program(MODE)
    nc = _program_cache[MODE]

    in_maps = make_in_maps(x, Wq, Wk, Wv)
    res = run_bass_kernel_spmd(nc, in_maps, list(range(N_CORES)))
    return np.stack([res.results[b]["out"] for b in range(BATCH)], axis=0)
